# revision 6
# baseline (speedup 1.0000x reference)
# Bidirectional LSTM layer (SEQ=512, BSZ=64, INP=HID=512) on 8 trn2 NeuronCores.
#
# Sharding: 2 directions x 4 batch-quarters -> 8 cores, no cross-core comm.
# Each core runs the SAME NEFF (SPMD); direction/batch differences are in data:
#   - backward cores get the input time-reversed (host un-reverses outputs).
#
# Per-core formulation (B=16 local batch):
#   Phase 1 (xproj): xg[t] = Wih_perm @ x[t].T + (bih+bhh)  for all t, batched
#     over time -> big stream-bound matmuls, spilled to DRAM.
#   Phase 2 (recurrence): for t: gates.T = xg[t] + Whh_perm @ h.T  with the
#     gate dim on partitions (transposed layout). Weights are the stationary
#     matmul operand in fp16 (enables Fast Weight Load); h is the moving
#     operand, also fp16; PSUM accumulates fp32; c stays fp32.
#   Gate row order is permuted to (i, g, f, o) so the last-arriving gate group
#   is o, minimizing the serial tail between steps.
#
# Layouts (device side, per core):
#   xT     [128, 4, T, 16] fp16   xT[p,k,t,b]    = x[t, b, k*128+p]
#   wih    [128, 4, 16, 128] fp16 wih[p,k,m,mi]  = Wp_ih[m*128+mi, k*128+p]
#   whh    [128, 4, 16, 128] fp16 (same, Wp_hh)
#   bias   [128, 16] fp32         bias[p,m]      = (bih+bhh)_perm[m*128+p]
#   h0     [128, 4, 16] fp16      h0[p,k,b]      = h0[b, k*128+p]
#   c0     [128, 4, 16] fp32
#   ysT    [128, T, 4, 16] fp32   out: ysT[p,t,k,b] = h_t[b, k*128+p]
#   cT     [128, 4, 16] fp32      out: final cell state
#   xg     [128, T+pad, 16, 16] fp32 (internal DRAM scratch)

import numpy as np

import concourse.bass as bass
import concourse.mybir as mybir
import concourse.tile as tile
from concourse import bacc
from concourse.bass import ds
from concourse.bass_utils import run_bass_kernel_spmd

SEQ, BSZ, INP, HID = 512, 64, 512, 512
B = 16          # local batch per core
KC = 4          # contraction chunks (512 / 128)
MC = 16         # gate-dim chunks (2048 / 128)
F16 = mybir.dt.float16
F32 = mybir.dt.float32

# gate row permutation: torch/jax order is (i, f, g, o); we reorder to
# (i, g, f, o) so 'o' arrives last and 'g' early.
GATE_PERM = np.r_[0:HID, 2 * HID:3 * HID, HID:2 * HID, 3 * HID:4 * HID]


def build_program(T=SEQ, unroll=16, xblk=32):
    assert T % unroll == 0 and unroll % 2 == 0
    assert T % xblk == 0
    half = unroll // 2
    iters = T // unroll
    Tpad = T + 2 * unroll

    nc = bacc.Bacc("TRN2", target_bir_lowering=False, debug=False)

    xT = nc.dram_tensor("xT", [128, KC, T, B], F16, kind="ExternalInput").ap()
    wih = nc.dram_tensor("wih", [128, KC, MC, 128], F16, kind="ExternalInput").ap()
    whh = nc.dram_tensor("whh", [128, KC, MC, 128], F16, kind="ExternalInput").ap()
    bias = nc.dram_tensor("bias", [128, MC], F32, kind="ExternalInput").ap()
    h0 = nc.dram_tensor("h0", [128, KC, B], F16, kind="ExternalInput").ap()
    c0 = nc.dram_tensor("c0", [128, KC, B], F32, kind="ExternalInput").ap()
    ysT = nc.dram_tensor("ysT", [128, T, KC, B], F32, kind="ExternalOutput").ap()
    cT = nc.dram_tensor("cT", [128, KC, B], F32, kind="ExternalOutput").ap()

    AF = mybir.ActivationFunctionType

    with tile.TileContext(nc) as tc:
        with (
            tc.tile_pool(name="const", bufs=1) as const,
            tc.tile_pool(name="dram", bufs=1, space="DRAM") as dpool,
            tc.tile_pool(name="state", bufs=1) as state,
            tc.tile_pool(name="rps", bufs=1, space="PSUM") as rps,
        ):
            wih_sb = const.tile([128, KC, MC, 128], F16, tag="wih")
            whh_sb = const.tile([128, KC, MC, 128], F16, tag="whh")
            bias_sb = const.tile([128, MC], F32, tag="bias")
            nc.sync.dma_start(out=wih_sb, in_=wih)
            nc.sync.dma_start(out=whh_sb, in_=whh)
            nc.sync.dma_start(out=bias_sb, in_=bias)

            xg_dram = dpool.tile([128, Tpad, MC, B], F32, tag="xg")
            # zero the prefetch-overrun pad so reads of it are defined
            with tc.tile_pool(name="zpad", bufs=1) as zpool:
                zt = zpool.tile([128, 2 * unroll, MC, B], F32, tag="zt")
                nc.vector.memset(zt, 0.0)
                nc.sync.dma_start(out=xg_dram[:, T:Tpad, :, :], in_=zt)

            # ---------------- Phase 1: input projections ----------------
            with (
                tc.tile_pool(name="xp_sb", bufs=3) as xp_sb,
                tc.tile_pool(name="xp_ps", bufs=4, space="PSUM") as xp_ps,
            ):
                for blk in range(T // xblk):
                    t0 = blk * xblk
                    xt = xp_sb.tile([128, KC, xblk, B], F16, tag="xt")
                    nc.sync.dma_start(out=xt, in_=xT[:, :, t0:t0 + xblk, :])
                    for m in range(MC):
                        ps = xp_ps.tile([128, xblk, B], F32, tag="xps")
                        for k in range(KC):
                            nc.tensor.matmul(
                                ps,
                                lhsT=wih_sb[:, k, m, :],
                                rhs=xt[:, k, :, :],
                                start=(k == 0),
                                stop=(k == KC - 1),
                            )
                        xgo = xp_sb.tile([128, xblk, B], F32, tag="xgo")
                        # xg = psum + bias (per-partition), fp32 out
                        nc.scalar.activation(
                            out=xgo, in_=ps, func=AF.Identity,
                            bias=bias_sb[:, m:m + 1], scale=1.0,
                        )
                        nc.sync.dma_start(
                            out=xg_dram[:, t0:t0 + xblk, m, :], in_=xgo
                        )

            # ---------------- Phase 2: recurrence ----------------
            # state tiles (ping-pong where needed)
            h_t = [state.tile([128, KC, B], F16, tag=f"h{j}", name=f"h{j}")
                   for j in range(2)]
            c_t = [state.tile([128, KC, B], F32, tag=f"c{j}", name=f"c{j}")
                   for j in range(2)]
            xg_buf = [state.tile([128, half, MC, B], F32, tag=f"xg{j}",
                                 name=f"xg{j}") for j in range(2)]
            ys_st = [state.tile([128, half, KC, B], F32, tag=f"ys{j}",
                                name=f"ys{j}") for j in range(2)]
            # per-step scratch (single set: reuse is safely serialized by deps)
            ps_ig = rps.tile([128, 8, B], F32, tag="ps_ig")   # i(m0-3) g(m4-7)
            ps_f = rps.tile([128, 4, B], F32, tag="ps_f")     # f(m8-11)
            ps_o = rps.tile([128, 4, B], F32, tag="ps_o")     # o(m12-15)
            gs_ig = state.tile([128, 8, B], F32, tag="gs_ig")
            gs_f = state.tile([128, 4, B], F32, tag="gs_f")
            gs_o = state.tile([128, 4, B], F32, tag="gs_o")
            t_si = state.tile([128, 4, B], F32, tag="t_si")
            t_tg = state.tile([128, 4, B], F32, tag="t_tg")
            t_sf = state.tile([128, 4, B], F32, tag="t_sf")
            t_so = state.tile([128, 4, B], F32, tag="t_so")
            t_ig = state.tile([128, 4, B], F32, tag="t_ig")
            t_fc = state.tile([128, 4, B], F32, tag="t_fc")
            t_tc = state.tile([128, 4, B], F32, tag="t_tc")

            # prologue
            nc.sync.dma_start(out=h_t[0], in_=h0)
            nc.sync.dma_start(out=c_t[0], in_=c0)
            nc.sync.dma_start(out=xg_buf[0], in_=xg_dram[:, 0:half, :, :])
            nc.sync.dma_start(out=xg_buf[1], in_=xg_dram[:, half:unroll, :, :])

            def step(u, xg_u):
                """One LSTM step. u: sub-step index in [0, unroll)."""
                cur, nxt = u % 2, (u + 1) % 2
                hcur, hnxt = h_t[cur], h_t[nxt]
                ccur, cnxt = c_t[cur], c_t[nxt]
                uh = u % half
                stg = ys_st[u // half]
                # matmuls: m-chunk groups i(0-3) g(4-7) -> ps_ig,
                # f(8-11) -> ps_f, o(12-15) -> ps_o
                for m in range(MC):
                    if m < 8:
                        out = ps_ig[:, m, :]
                    elif m < 12:
                        out = ps_f[:, m - 8, :]
                    else:
                        out = ps_o[:, m - 12, :]
                    for k in range(KC):
                        nc.tensor.matmul(
                            out,
                            lhsT=whh_sb[:, k, m, :],
                            rhs=hcur[:, k, :],
                            start=(k == 0),
                            stop=(k == KC - 1),
                        )
                    if m == 7:
                        nc.vector.tensor_add(gs_ig, ps_ig, xg_u[:, 0:8, :])
                        nc.scalar.activation(
                            out=t_si, in_=gs_ig[:, 0:4, :], func=AF.Sigmoid)
                        nc.scalar.activation(
                            out=t_tg, in_=gs_ig[:, 4:8, :], func=AF.Tanh)
                        nc.vector.tensor_mul(t_ig, t_si, t_tg)
                    elif m == 11:
                        nc.vector.tensor_add(gs_f, ps_f, xg_u[:, 8:12, :])
                        nc.scalar.activation(
                            out=t_sf, in_=gs_f, func=AF.Sigmoid)
                        nc.vector.tensor_mul(t_fc, t_sf, ccur)
                        nc.vector.tensor_add(cnxt, t_ig, t_fc)
                        nc.scalar.activation(out=t_tc, in_=cnxt, func=AF.Tanh)
                # o arrives last: short tail
                nc.vector.tensor_add(gs_o, ps_o, xg_u[:, 12:16, :])
                nc.scalar.activation(out=t_so, in_=gs_o, func=AF.Sigmoid)
                nc.vector.tensor_mul(hnxt, t_so, t_tc)      # fp16 h for next MM
                nc.vector.tensor_mul(stg[:, uh, :, :], t_so, t_tc)  # fp32 out

            with tc.For_i(0, iters, 1, hint_engines=(mybir.EngineType.PE,)) as i:
                t0 = i * unroll
                for u in range(half):
                    step(u, xg_buf[0][:, u, :, :])
                nc.sync.dma_start(
                    out=ysT[:, ds(t0, half), :, :], in_=ys_st[0])
                nc.sync.dma_start(
                    out=xg_buf[0],
                    in_=xg_dram[:, ds(t0 + unroll, half), :, :])
                for u in range(half, unroll):
                    step(u, xg_buf[1][:, u - half, :, :])
                nc.sync.dma_start(
                    out=ysT[:, ds(t0 + half, half), :, :], in_=ys_st[1])
                nc.sync.dma_start(
                    out=xg_buf[1],
                    in_=xg_dram[:, ds(t0 + unroll + half, half), :, :])

            # final cell state: step T-1 wrote c_t[(T-1+1) % 2] = c_t[T % 2]
            nc.sync.dma_start(out=cT, in_=c_t[T % 2])

    nc.compile()
    return nc


# ---------------- host-side staging ----------------

def _stage_weights(W_ih, W_hh, b_ih, b_hh):
    Wp_ih = W_ih[GATE_PERM]          # [2048, 512]
    Wp_hh = W_hh[GATE_PERM]
    bp = (b_ih + b_hh)[GATE_PERM]    # [2048]
    wih = np.ascontiguousarray(
        Wp_ih.T.reshape(KC, 128, MC, 128).transpose(1, 0, 2, 3)
    ).astype(np.float16)
    whh = np.ascontiguousarray(
        Wp_hh.T.reshape(KC, 128, MC, 128).transpose(1, 0, 2, 3)
    ).astype(np.float16)
    bias = np.ascontiguousarray(bp.reshape(MC, 128).T).astype(np.float32)
    return wih, whh, bias


def _stage_x(x_q):
    # x_q: [T, B, INP] (already batch-sliced, already time-reversed for bwd)
    T = x_q.shape[0]
    xt = x_q.transpose(2, 0, 1).reshape(KC, 128, T, B).transpose(1, 0, 2, 3)
    return np.ascontiguousarray(xt).astype(np.float16)


def _stage_state(s_q, dtype):
    # s_q: [B, HID] -> [128, KC, B]
    st = s_q.T.reshape(KC, 128, B).transpose(1, 0, 2)
    return np.ascontiguousarray(st).astype(dtype)


def _unstage_ys(ysT):
    # ysT: [128, T, KC, B] -> [T, B, HID]
    T = ysT.shape[1]
    return np.ascontiguousarray(
        ysT.transpose(1, 3, 2, 0).reshape(T, B, HID))


def _unstage_state(sT):
    # [128, KC, B] -> [B, HID]
    return np.ascontiguousarray(sT.transpose(2, 1, 0).reshape(B, HID))


def stage_core_inputs(inputs, direction, q, T=SEQ):
    """direction: 0=forward, 1=backward; q: batch quarter index."""
    d = "f" if direction == 0 else "b"
    bs = slice(q * B, (q + 1) * B)
    wih, whh, bias = _stage_weights(
        inputs[f"W_ih_{d}"], inputs[f"W_hh_{d}"],
        inputs[f"b_ih_{d}"], inputs[f"b_hh_{d}"])
    x_q = inputs["input_"][:T, bs, :]
    if direction == 1:
        x_q = x_q[::-1]
    return {
        "xT": _stage_x(x_q),
        "wih": wih,
        "whh": whh,
        "bias": bias,
        "h0": _stage_state(inputs[f"h0_{d}"][bs], np.float16),
        "c0": _stage_state(inputs[f"c0_{d}"][bs], np.float32),
    }


def run_cores(inputs, T=SEQ, unroll=16, xblk=32, trace=False, n_cores=8):
    nc = build_program(T=T, unroll=unroll, xblk=xblk)
    in_maps = [
        stage_core_inputs(inputs, c // 4, c % 4, T=T) for c in range(n_cores)
    ]
    res = run_bass_kernel_spmd(
        nc, in_maps, core_ids=list(range(n_cores)), trace=trace)
    return res


def assemble(results, inputs, T=SEQ):
    out = np.zeros((T, BSZ, 2 * HID), np.float32)
    cT_f = np.zeros((BSZ, HID), np.float32)
    cT_b = np.zeros((BSZ, HID), np.float32)
    for c, r in enumerate(results):
        direction, q = c // 4, c % 4
        bs = slice(q * B, (q + 1) * B)
        ys = _unstage_ys(r["ysT"])
        cc = _unstage_state(r["cT"])
        if direction == 0:
            out[:, bs, :HID] = ys
            cT_f[bs] = cc
        else:
            out[:, bs, HID:] = ys[::-1]
            cT_b[bs] = cc
    hT_f = out[T - 1, :, :HID].copy()
    hT_b = out[0, :, HID:].copy()
    return out, hT_f, cT_f, hT_b, cT_b


def kernel(**inputs):
    inputs = {k: np.asarray(v) for k, v in inputs.items()}
    res = run_cores(inputs)
    return assemble(res.results, inputs)
